# revision 1
# baseline (speedup 1.0000x reference)
"""Trainium2 Bass kernel for CustomApplyTimeChannel.

Computes, per (batch b, block n):
    y[b,n,:] = full_conv(x[b,n,:1096], h[b,n,:24])   # length 1119
then overlap-adds frames with hop T=1096 into out[b, :15367].

Sharding: pure data parallel over batch across 8 NeuronCores (16 b/core).

Per-core layout: 224 (n,b) rows, n-major (p = n*16 + b), split into two
partition tiles (128 + 96).  The 24 taps are split across three engines,
all fp32-exact:
  - tap 0 on ScalarE: Y[:, 0:T] = x * h[:, 0]  (activation scale is [P,1])
  - taps 1..NPE on TensorE: PSUM_Y += diag(h[:, j]) @ Xpad_shifted, where
    the diagonal weight is built on ScalarE as eye * h[:, j] and Xpad is
    zero-padded so every tap streams the full 1119 output columns
  - taps NPE+1..23 on VectorE as fused MACs:
    Y[:, j:j+T] = (x * h[:, j]) + Y[:, j:j+T]   (scalar_tensor_tensor)
then Y += PSUM_Y.  The overlap-add between frame n-1's tail and frame n's
head is a partition shift by +16, done with three small SBUF->SBUF DMAs
and two 23-wide adds; the last frame's tail is stored directly.

This container's walrus compiler accepts only ONE sync wait per
instruction; see _patch_drain_split/_audit_single_wait and the "join"
copies below for how the kernel is kept within that limit.
"""

import sys

sys.path.insert(0, "/opt/trn_rl_repo")

import numpy as np

from concourse import bass, tile
from concourse import mybir
from concourse.bass_utils import run_bass_kernel_spmd

# Problem constants (hardcoded; kernel.py must be self-contained).
B = 128          # total batch
NB = 14          # channel blocks
T = 1096         # time samples per block
L = 24           # taps
F = T + L - 1    # frame length 1119
OUT_LEN = (NB - 1) * T + F   # 15367
N_CORES = 8
BC = B // N_CORES            # 16 batches per core
ROWS = NB * BC               # 224 rows per core
P0 = 128                     # tile0 partitions (n in 0..7)
P1 = ROWS - P0               # tile1 partitions = 96 (n in 8..13)

FP32 = mybir.dt.float32

# Taps 1..NPE (per tile) run on the TensorE as diag-weight matmuls
# accumulating in PSUM; taps NPE+1..23 run on DVE as fused STT MACs.
NPE = 12
WX = 23 + T + 33  # padded x width (1152): 23 left zeros, 33 right zeros

_CACHE = {}


def _patch_drain_split():
    """The walrus build here allows ONE sync wait per instruction; Tile's
    kernel-tail drain carries one wait per outstanding processor.  Split the
    drain into a chain of single-wait drains (same position, same sems)."""
    if getattr(tile.TileContext, "_drain_split_patched", False):
        return
    from concourse.vector_clock import ScopedClock

    def _drain_and_barrier(self, tick_clock, wait_clock):
        drain_inst = self.nc.sync.drain()
        wait_clock.add_sem_waits(
            drain_inst.ins, ScopedClock({None: tick_clock.global_clock})
        )
        si = drain_inst.ins.sync_info
        if si is not None and len(si.on_wait) > 1:
            waits = list(si.on_wait)
            drain_inst.ins.sync_info = mybir.SyncInfo(
                on_wait=[waits[0]], on_update=list(si.on_update)
            )
            for w in waits[1:]:
                d2 = self.nc.sync.drain()
                d2.ins.sync_info = mybir.SyncInfo(on_wait=[w], on_update=[])
        self.nc.all_engine_barrier()
        popped = self.nc._tile_sem_poison_stack.pop()
        assert popped is self._sem_poison
        self.nc.clear_and_free_semaphores(list(self.sems.allocated().values()))
        self.nc.all_engine_barrier()

    tile.TileContext._drain_and_barrier = _drain_and_barrier
    tile.TileContext._drain_split_patched = True


_ENG_SEM_PREFIX = {
    mybir.EngineType.PE: "PE_",
    mybir.EngineType.DVE: "DVE_",
    mybir.EngineType.Activation: "Activation_",
    mybir.EngineType.Pool: "Pool_",
    mybir.EngineType.SP: "SP_",
}


def _drop_self_waits(nc):
    """An engine's instruction stream executes in order, so a wait on the
    instruction's own engine semaphore is redundant.  Drop those when an
    instruction carries more than the single wait the ISA slot allows."""
    for f in nc.m.functions:
        for blk in f.blocks:
            for ins in blk.instructions:
                si = ins.sync_info
                if si is None or len(si.on_wait) <= 1:
                    continue
                pref = _ENG_SEM_PREFIX.get(ins.engine)
                if pref is None:
                    continue
                keep = [w for w in si.on_wait if not (w.ant_name or "").startswith(pref)]
                if len(keep) < len(si.on_wait) and len(keep) <= 1:
                    ins.sync_info = mybir.SyncInfo(
                        on_wait=keep, on_update=list(si.on_update)
                    )


def _audit_single_wait(nc):
    bad = []
    for f in nc.m.functions:
        for blk in f.blocks:
            for ins in blk.instructions:
                si = ins.sync_info
                if si is not None and len(si.on_wait) > 1:
                    bad.append((type(ins).__name__, ins.name, len(si.on_wait)))
    if bad:
        raise RuntimeError(f"instructions with >1 sync wait: {bad}")


def _build_nc():
    _patch_drain_split()
    nc = bass.Bass()
    x_ext = nc.declare_dram_parameter("x", [BC, NB, T], FP32, isOutput=False)
    h_ext = nc.declare_dram_parameter("h", [BC, NB, L], FP32, isOutput=False)
    if NPE > 0:
        eye_ext = nc.declare_dram_parameter("eye", [P0, P0], FP32, isOutput=False)
    out_ext = nc.declare_dram_parameter("out", [BC, OUT_LEN], FP32, isOutput=True)

    # Row-major over (n, b): row p = n*BC + b.  Expressed as 3D (n, b, t)
    # APs on the DRAM side, matched to 3D views of the SBUF tiles.
    xv = x_ext.transpose([1, 0, 2])                    # [NB, BC, T]
    hv = h_ext.transpose([1, 0, 2])                    # [NB, BC, L]
    # Main output region: out[b, n*T + t] for t < T, as (n, b, t).
    ov = (
        out_ext[:, : NB * T]
        .rearrange("b (n t) -> b n t", n=NB, t=T)
        .transpose([1, 0, 2])
    )

    with tile.TileContext(nc) as tc:
        with (
            tc.tile_pool(name="main", bufs=1) as pool,
            tc.tile_pool(name="diag", bufs=3) as dgpool,
            tc.tile_pool(name="ps", bufs=1, space="PSUM") as pspool,
        ):
            SCR = pool.tile([64, 16], FP32, tag="scr")
            SCR2 = pool.tile([64, 16], FP32, tag="scr2")
            X0 = pool.tile([P0, WX], FP32, tag="x0")
            X1 = pool.tile([P1, WX], FP32, tag="x1")
            H0 = pool.tile([P0, L], FP32, tag="h0")
            H1 = pool.tile([P1, L], FP32, tag="h1")
            Y0 = pool.tile([P0, F], FP32, tag="y0")
            Y1 = pool.tile([P1, F], FP32, tag="y1")
            T0 = pool.tile([P0, L - 1], FP32, tag="t0")
            T1 = pool.tile([P1, L - 1], FP32, tag="t1")
            X, H, Y, TLS = [X0, X1], [H0, H1], [Y0, Y1], [T0, T1]
            PN = [P0, P1]
            if NPE > 0:
                EY = pool.tile([P0, P0], FP32, tag="eye")
                YP0 = pspool.tile([P0, F], FP32, tag="yp0")
                YP1 = pspool.tile([P1, F], FP32, tag="yp1")
                PSCR = pspool.tile([1, 8], FP32, tag="pscr")
                YP = [YP0, YP1]

            def xs(k):  # the unpadded x window
                return X[k][:, 23 : 23 + T]

            # SBUF-side APs stay 2D (Tile's dep tracking mishandles
            # partition-split views); all reordering lives on the DRAM side.
            NS = [(0, 8), (8, NB)]  # n-ranges per tile
            for k, (n0, n1) in enumerate(NS):
                nc.sync.dma_start(out=X[k][:, 23 : 23 + T], in_=xv[n0:n1])
                nc.sync.dma_start(out=H[k][:], in_=hv[n0:n1])
                # zero pads around x (DVE)
                nc.vector.memset(X[k][:, 0:23], 0.0)
                nc.vector.memset(X[k][:, 23 + T : WX], 0.0)
            if NPE > 0:
                nc.scalar.dma_start(out=EY[:], in_=eye_ext[:])

            # The engine ISA structs hold only ONE sync wait, so no compute
            # op may depend on two DMA queues at once.  Tiny "join" copies
            # absorb each DMA wait into the engine's vector clock first.
            _jc = [0]

            def join(src_tile, pb, col=0, eng=nc.vector, scr=None):
                i = _jc[0]
                _jc[0] += 1
                scr = SCR if scr is None else scr
                dst = scr[pb : pb + 1, i : i + 1]
                src = src_tile[pb : pb + 1, col : col + 1]
                if eng is nc.vector:
                    nc.vector.tensor_copy(dst, src)
                else:
                    nc.scalar.copy(dst, src)

            for k in range(2):
                join(X[k], 0, col=500)             # DVE observes x-DMA
                join(H[k], 0)                      # DVE observes h-DMA
            if NPE > 0:
                for k in range(2):
                    join(X[k], 0, col=500, eng=nc.scalar, scr=SCR2)
                    join(H[k], 0, eng=nc.scalar, scr=SCR2)
                join(EY, 0, eng=nc.scalar, scr=SCR2)

            # Tap 0 initializes Y[:, 0:T] (ScalarE); DVE memsets the tail.
            for k in range(2):
                nc.scalar.mul(Y[k][:, 0:T], xs(k), H[k][:, 0:1])
                nc.vector.memset(Y[k][:, T:F], 0.0)
            for k in range(2):
                # DVE observes the ACT tap-0 writes before the STT taps.
                join(Y[k], 0)

            if NPE > 0:
                # PE joins: dummy 1x1 matmuls absorbing the Xpad deps so the
                # real matmuls carry at most the one ACT (diag) wait.
                for i, (tl, c) in enumerate(
                    [(X0, 0), (X0, 500), (X1, 0), (X1, 500)]
                ):
                    cell = tl[0:1, c : c + 1]
                    nc.tensor.matmul(
                        PSCR[0:1, i : i + 1], cell, cell, start=True, stop=True
                    )
                # Taps 1..NPE on PE: Y_psum += diag(h_j) @ Xpad shifted views.
                PIECES = [(0, 512), (512, 1024), (1024, F)]
                for k in range(2):
                    for j in range(1, NPE + 1):
                        # distinct tile per tap: no slot-reuse WAR waits
                        DG = dgpool.tile([PN[k], PN[k]], FP32, tag=f"dg{k}_{j}")
                        nc.scalar.mul(DG[:], EY[0 : PN[k], 0 : PN[k]], H[k][:, j : j + 1])
                        for c0, c1 in PIECES:
                            nc.tensor.matmul(
                                YP[k][:, c0:c1],
                                DG[:],
                                X[k][:, 23 - j + c0 : 23 - j + c1],
                                start=(j == 1),
                                stop=(j == NPE),
                            )

            # Remaining taps on DVE as fused scalar*tensor+tensor MACs.
            for j in range(NPE + 1, L):
                for k in range(2):
                    nc.vector.scalar_tensor_tensor(
                        out=Y[k][:, j : j + T],
                        in0=xs(k),
                        scalar=H[k][:, j : j + 1],
                        in1=Y[k][:, j : j + T],
                        op0=mybir.AluOpType.mult,
                        op1=mybir.AluOpType.add,
                    )
            if NPE > 0:
                # Fold the PE partial sums into Y (DVE first observes the PE
                # clock through a one-cell PSUM read).
                for k in range(2):
                    join(YP[k], 0)
                for k in range(2):
                    nc.vector.tensor_add(Y[k][:], Y[k][:], YP[k][:])

            # POOL observes the ACT clock via a pure-ACT artifact before the
            # tail DMAs trigger, so those keep only their DVE wait (Tile's
            # DMA-side dependency tracking is tile-coarse and would otherwise
            # add a spurious ACT wait).
            SCRP = pool.tile([1, 8], FP32, tag="scrp")
            if NPE > 0:
                nc.gpsimd.tensor_copy(SCRP[0:1, 0:1], DG[0:1, 0:1])
            else:
                nc.scalar.copy(SCR2[0:1, 15:16], SCR2[0:1, 0:1])
                nc.gpsimd.tensor_copy(SCRP[0:1, 0:1], SCR2[0:1, 15:16])

            # Overlap-add: row p (= n*BC + b) with n >= 1 needs the tail of
            # row p - BC added to its head.  Shift tails down 16 partitions.
            nc.vector.memset(TLS[0][0:BC, :], 0.0)          # n = 0: no tail
            nc.gpsimd.dma_start(out=TLS[0][BC:P0, :], in_=Y[0][0 : P0 - BC, T:F])
            nc.gpsimd.dma_start(out=TLS[1][0:BC, :], in_=Y[0][P0 - BC : P0, T:F])
            nc.gpsimd.dma_start(out=TLS[1][BC:P1, :], in_=Y[1][0 : P1 - BC, T:F])
            # In-place one-cell copies on the TLS tiles: absorb each tail
            # DMA's queue semaphore into DVE AND create a write the adds
            # depend on, forcing join-before-add scheduling.
            nc.vector.tensor_copy(TLS[0][32:33, 0:1], TLS[0][32:33, 0:1])
            nc.vector.tensor_copy(TLS[1][0:1, 1:2], TLS[1][0:1, 1:2])
            nc.vector.tensor_copy(TLS[1][32:33, 1:2], TLS[1][32:33, 1:2])
            for k in range(2):
                nc.vector.tensor_add(Y[k][:, 0 : L - 1], Y[k][:, 0 : L - 1], TLS[k][:])

            # Store main frames and the final tail (last frame's spill), on
            # the POOL descriptor ring (1 wait each: the DVE completion).
            for k, (n0, n1) in enumerate(NS):
                nc.gpsimd.dma_start(out=ov[n0:n1], in_=Y[k][:, 0:T])
            nc.gpsimd.dma_start(
                out=out_ext[:, NB * T : OUT_LEN],
                in_=Y[1][P1 - BC : P1, T:F],
            )
    _audit_single_wait(nc)
    return nc


def _get_nc():
    if "nc" not in _CACHE:
        _CACHE["nc"] = _build_nc()
    return _CACHE["nc"]


def _run(x, h_time, trace=False, **kw):
    x = np.ascontiguousarray(np.asarray(x, dtype=np.float32))
    h = np.ascontiguousarray(np.asarray(h_time, dtype=np.float32))
    nc = _get_nc()
    eye = np.eye(P0, dtype=np.float32)
    in_maps = [
        {"x": x[i * BC : (i + 1) * BC], "h": h[i * BC : (i + 1) * BC]}
        for i in range(N_CORES)
    ]
    if NPE > 0:
        for m in in_maps:
            m["eye"] = eye
    res = run_bass_kernel_spmd(nc, in_maps, list(range(N_CORES)), trace=trace, **kw)
    out = np.concatenate([res.results[i]["out"] for i in range(N_CORES)], axis=0)
    return out.astype(np.float32), res


def kernel(x, h_time):
    out, _ = _run(x, h_time, trace=False)
    return out


if __name__ == "__main__":
    # Dry build: just construct the program and report instruction counts.
    nc = _build_nc()
    from collections import Counter

    cnt = Counter()
    for f in nc.m.functions:
        for blk in f.blocks:
            for ins in blk.instructions:
                cnt[type(ins).__name__] += 1
    print(dict(cnt))
    print("total instructions:", sum(cnt.values()))



# revision 8
# speedup vs baseline: 112.2850x; 112.2850x over previous
"""Trainium2 Bass kernel for CustomApplyTimeChannel.

Computes, per (batch b, block n):
    y[b,n,:] = full_conv(x[b,n,:1096], h[b,n,:24])   # length 1119
then overlap-adds frames with hop T=1096 into out[b, :15367].

Sharding: pure data parallel over batch across 8 NeuronCores (16 b/core).

Per-core layout: 224 (n,b) rows, n-major (p = n*16 + b), split into two
partition tiles (128 + 96).  Compute is deliberately minimal-structure —
only ACT + DVE + HWDGE DMA rings (no PE, no PSUM, no GPSIMD/SWDGE):
  - tap 0 on ScalarE: Y[:, 0:T] = x * h[:, 0]  (activation scale is [P,1])
  - taps 1..23 on VectorE as fused MACs:
    Y[:, j:j+T] = (x * h[:, j]) + Y[:, j:j+T]   (scalar_tensor_tensor)
The overlap-add between frame n-1's tail and frame n's head is a partition
shift by +16, done with three small SBUF->SBUF DMAs on the ACT HWDGE ring
and two 23-wide DVE adds; the last frame's tail is stored directly.

This container's walrus compiler accepts only ONE sync wait per
instruction; see _patch_drain_split/_audit_single_wait and the "join"
copies below for how the kernel is kept within that limit.
"""

import sys

sys.path.insert(0, "/opt/trn_rl_repo")

import numpy as np

from concourse import bass, tile
from concourse import mybir
from concourse.bass_utils import run_bass_kernel_spmd

# Problem constants (hardcoded; kernel.py must be self-contained).
B = 128          # total batch
NB = 14          # channel blocks
T = 1096         # time samples per block
L = 24           # taps
F = T + L - 1    # frame length 1119
OUT_LEN = (NB - 1) * T + F   # 15367
N_CORES = 8
BC = B // N_CORES            # 16 batches per core
ROWS = NB * BC               # 224 rows per core
P0 = 128                     # tile0 partitions (n in 0..7)
P1 = ROWS - P0               # tile1 partitions = 96 (n in 8..13)

FP32 = mybir.dt.float32

_CACHE = {}


def _patch_drain_split():
    """The walrus build here allows ONE sync wait per instruction; Tile's
    kernel-tail drain carries one wait per outstanding processor.  Split the
    drain into a chain of single-wait drains (same position, same sems)."""
    if getattr(tile.TileContext, "_drain_split_patched", False):
        return
    from concourse.vector_clock import ScopedClock

    def _drain_and_barrier(self, tick_clock, wait_clock):
        drain_inst = self.nc.sync.drain()
        wait_clock.add_sem_waits(
            drain_inst.ins, ScopedClock({None: tick_clock.global_clock})
        )
        si = drain_inst.ins.sync_info
        if si is not None and len(si.on_wait) > 1:
            waits = list(si.on_wait)
            drain_inst.ins.sync_info = mybir.SyncInfo(
                on_wait=[waits[0]], on_update=list(si.on_update)
            )
            for w in waits[1:]:
                d2 = self.nc.sync.drain()
                d2.ins.sync_info = mybir.SyncInfo(on_wait=[w], on_update=[])
        self.nc.all_engine_barrier()
        popped = self.nc._tile_sem_poison_stack.pop()
        assert popped is self._sem_poison
        self.nc.clear_and_free_semaphores(list(self.sems.allocated().values()))
        self.nc.all_engine_barrier()

    tile.TileContext._drain_and_barrier = _drain_and_barrier
    tile.TileContext._drain_split_patched = True


_ENG_SEM_PREFIX = {
    mybir.EngineType.PE: "PE_",
    mybir.EngineType.DVE: "DVE_",
    mybir.EngineType.Activation: "Activation_",
    mybir.EngineType.Pool: "Pool_",
    mybir.EngineType.SP: "SP_",
}


def _drop_self_waits(nc):
    """An engine's instruction stream executes in order, so a wait on the
    instruction's own engine semaphore is redundant.  Drop those when an
    instruction carries more than the single wait the ISA slot allows."""
    for f in nc.m.functions:
        for blk in f.blocks:
            for ins in blk.instructions:
                si = ins.sync_info
                if si is None or len(si.on_wait) <= 1:
                    continue
                pref = _ENG_SEM_PREFIX.get(ins.engine)
                if pref is None:
                    continue
                keep = [w for w in si.on_wait if not (w.ant_name or "").startswith(pref)]
                if len(keep) < len(si.on_wait) and len(keep) <= 1:
                    ins.sync_info = mybir.SyncInfo(
                        on_wait=keep, on_update=list(si.on_update)
                    )


def _audit_single_wait(nc):
    bad = []
    for f in nc.m.functions:
        for blk in f.blocks:
            for ins in blk.instructions:
                si = ins.sync_info
                if si is not None and len(si.on_wait) > 1:
                    bad.append((type(ins).__name__, ins.name, len(si.on_wait)))
    if bad:
        raise RuntimeError(f"instructions with >1 sync wait: {bad}")


def _build_nc():
    _patch_drain_split()
    nc = bass.Bass()
    x_ext = nc.declare_dram_parameter("x", [BC, NB, T], FP32, isOutput=False)
    h_ext = nc.declare_dram_parameter("h", [BC, NB, L], FP32, isOutput=False)
    out_ext = nc.declare_dram_parameter("out", [BC, OUT_LEN], FP32, isOutput=True)

    # Row-major over (n, b): row p = n*BC + b.  Expressed as 3D (n, b, t)
    # APs on the DRAM side, matched to 2D SBUF tiles.
    xv = x_ext.transpose([1, 0, 2])                    # [NB, BC, T]
    hv = h_ext.transpose([1, 0, 2])                    # [NB, BC, L]
    # Main output region: out[b, n*T + t] for t < T, as (n, b, t).
    ov = (
        out_ext[:, : NB * T]
        .rearrange("b (n t) -> b n t", n=NB, t=T)
        .transpose([1, 0, 2])
    )
    NS = [(0, 8), (8, NB)]  # n-ranges per tile

    with tile.TileContext(nc) as tc:
        with tc.tile_pool(name="main", bufs=1) as pool:
            SCR = pool.tile([64, 16], FP32, tag="scr")
            SCR2 = pool.tile([64, 16], FP32, tag="scr2")
            X0 = pool.tile([P0, T], FP32, tag="x0")
            X1 = pool.tile([P1, T], FP32, tag="x1")
            H0 = pool.tile([P0, L], FP32, tag="h0")
            H1 = pool.tile([P1, L], FP32, tag="h1")
            Y0 = pool.tile([P0, F], FP32, tag="y0")
            Y1 = pool.tile([P1, F], FP32, tag="y1")
            T0 = pool.tile([P0, L - 1], FP32, tag="t0")
            T1 = pool.tile([P1, L - 1], FP32, tag="t1")
            X, H, Y, TLS = [X0, X1], [H0, H1], [Y0, Y1], [T0, T1]

            for k, (n0, n1) in enumerate(NS):
                nc.sync.dma_start(out=X[k][:], in_=xv[n0:n1])
                nc.sync.dma_start(out=H[k][:], in_=hv[n0:n1])

            # The engine ISA structs hold only ONE sync wait, so no compute
            # op may depend on two DMA queues at once.  Tiny "join" copies
            # absorb each DMA wait into the engine's vector clock first.
            _jc = [0]

            def join(src_tile, eng=None, scr=None):
                i = _jc[0]
                _jc[0] += 1
                scr = SCR if scr is None else scr
                dst = scr[0:1, i : i + 1]
                src = src_tile[0:1, 0:1]
                if eng is None:
                    nc.vector.tensor_copy(dst, src)
                else:
                    nc.scalar.copy(dst, src)

            # DVE and ACT observe the four input DMAs (one wait each).
            for k in range(2):
                join(X[k])
                join(H[k])
            for k in range(2):
                join(X[k], eng="act", scr=SCR2)
                join(H[k], eng="act", scr=SCR2)

            # n = 0 rows have no predecessor tail.
            nc.vector.memset(TLS[0][0:BC, :], 0.0)
            # Tap 0 initializes Y[:, 0:T] (ScalarE); DVE memsets the tail.
            for k in range(2):
                nc.scalar.mul(Y[k][:, 0:T], X[k][:], H[k][:, 0:1])
                nc.vector.memset(Y[k][:, T:F], 0.0)
            for k in range(2):
                join(Y[k])  # DVE observes the ACT tap-0 writes

            # Taps 1..23 on DVE as fused scalar*tensor+tensor MACs.
            for j in range(1, L):
                for k in range(2):
                    nc.vector.scalar_tensor_tensor(
                        out=Y[k][:, j : j + T],
                        in0=X[k][:],
                        scalar=H[k][:, j : j + 1],
                        in1=Y[k][:, j : j + T],
                        op0=mybir.AluOpType.mult,
                        op1=mybir.AluOpType.add,
                    )

            # Overlap-add: row p (= n*BC + b) with n >= 1 needs the tail of
            # row p - BC added to its head.  Shift tails down 16 partitions
            # with SBUF->SBUF DMAs on the ACT HWDGE ring (ACT wrote Y's tap-0
            # region itself, so only the DVE wait survives on each).
            nc.scalar.dma_start(out=TLS[0][BC:P0, :], in_=Y[0][0 : P0 - BC, T:F])
            nc.scalar.dma_start(out=TLS[1][0:BC, :], in_=Y[0][P0 - BC : P0, T:F])
            nc.scalar.dma_start(out=TLS[1][BC:P1, :], in_=Y[1][0 : P1 - BC, T:F])
            # In-place one-cell copies on the TLS tiles: absorb each tail
            # DMA's queue semaphore into DVE AND create a write the adds
            # depend on, forcing join-before-add scheduling.
            nc.vector.tensor_copy(TLS[0][32:33, 0:1], TLS[0][32:33, 0:1])
            nc.vector.tensor_copy(TLS[1][0:1, 1:2], TLS[1][0:1, 1:2])
            nc.vector.tensor_copy(TLS[1][32:33, 1:2], TLS[1][32:33, 1:2])
            for k in range(2):
                nc.vector.tensor_add(Y[k][:, 0 : L - 1], Y[k][:, 0 : L - 1], TLS[k][:])

            # Store main frames and the final tail (last frame's spill) on
            # the ACT HWDGE ring (1 wait each: the DVE completion).
            for k, (n0, n1) in enumerate(NS):
                nc.scalar.dma_start(out=ov[n0:n1], in_=Y[k][:, 0:T])
            nc.scalar.dma_start(
                out=out_ext[:, NB * T : OUT_LEN],
                in_=Y[1][P1 - BC : P1, T:F],
            )
    _drop_self_waits(nc)
    _audit_single_wait(nc)
    return nc


def _get_nc():
    if "nc" not in _CACHE:
        _CACHE["nc"] = _build_nc()
    return _CACHE["nc"]


def _in_maps(x, h):
    return [
        {"x": x[i * BC : (i + 1) * BC], "h": h[i * BC : (i + 1) * BC]}
        for i in range(N_CORES)
    ]


def _run(x, h_time, trace=False, **kw):
    x = np.ascontiguousarray(np.asarray(x, dtype=np.float32))
    h = np.ascontiguousarray(np.asarray(h_time, dtype=np.float32))
    nc = _get_nc()
    res = run_bass_kernel_spmd(nc, _in_maps(x, h), list(range(N_CORES)), trace=trace, **kw)
    out = np.concatenate([res.results[i]["out"] for i in range(N_CORES)], axis=0)
    return out.astype(np.float32), res


def kernel(x, h_time):
    out, _ = _run(x, h_time, trace=False)
    return out


if __name__ == "__main__":
    # Dry build: just construct the program and report instruction counts.
    nc = _build_nc()
    from collections import Counter

    cnt = Counter()
    for f in nc.m.functions:
        for blk in f.blocks:
            for ins in blk.instructions:
                cnt[type(ins).__name__] += 1
    print(dict(cnt))
    print("total instructions:", sum(cnt.values()))
